# revision 3
# baseline (speedup 1.0000x reference)
"""Triangular matmul C = triu(triu(A) @ triu(B)) on 8 TRN2 NeuronCores.

Structure: the (I, K, J) block-tetrahedron {I <= K <= J} (128x128 blocks,
N=4096 -> 32 blocks/side) is sharded by output row-block I across the 8
cores with a work-balanced assignment.  Each core runs its own statically
addressed program inside a `tc.If(partition_id == c)` block.

Per core: row-blocks are processed in groups of <=4 that share one sweep
over the B strips (B[K, K*128:] for K >= min(group)).  The J axis is cut
into 512-wide phases; each group member I gets one PSUM bank per phase
(double-buffered), accumulating A^T[K,I] @ B[K, phase-window] over K, then
evicting to the output.

Numerics: fp32 operands are split on the host into bf16 (hi, lo) pairs and
each block product uses 3 bf16 matmuls (Ah@Bh + Ah@Bl + Al@Bh), giving
~5e-6 relative error vs the fp32 reference at 3/4 the cost of the PE's
native 4-pass fp32 mode (measured on HW: rel_absmax 4.8e-6 at N=4096).

DMA discipline: one ~256KB DMA per (K-strip, phase) carrying hi and lo
planes together (Bcat = [Bh | Bl]), issued alternately from the SP and DVE
sequencers; A^T strips are host-packed per core (apack) so each strip is a
few large-line DMAs on the GpSimd sequencer; PSUM evictions copy on ACT and
store from its sequencer.  This keeps ~16 DMA engines busy without
sequencer issue serialization.

The kernel takes FULL (unsharded) inputs and returns the FULL output.
"""

import numpy as np

N = 4096
BLK = 128
NB = N // BLK  # 32
N_CORES = 8
PHASE = 512  # J-phase width (one PSUM bank of fp32)
MODE = "bf16x3"  # "bf16x3" | "fp32r" | "fp32"

# Work-balanced assignment of row-blocks I to cores (work(I) = T(32-I),
# T(m)=m(m+1)/2; bins balanced to 743..752 of 5984/8=748).
BINS = [
    [0, 14, 23],
    [1, 15, 21, 25, 29],
    [2, 13, 20, 28],
    [4, 12, 16],
    [3, 10, 22],
    [6, 9, 17, 30],
    [5, 11, 19, 24, 27, 31],
    [7, 8, 18, 26],
]
MAXB = max(len(b) for b in BINS)  # output row-slots per core
# A-pack slot layout: per core, the A^T strips (one 128x128 block per slot,
# hi+lo planes) for each owned I, K = I..31, concatenated.
ABASE = [
    {I: int(np.cumsum([0] + [NB - J for J in sorted(b)])[i]) for i, I in enumerate(sorted(b))}
    for b in BINS
]
NSLOT = 80  # >= max per-core total blocks (75)
ACHUNK = 4  # A-load DMA granularity in k-blocks


def _groups(bin_is):
    """Split a sorted bin into contiguous groups of <=4 minimizing the
    total B-strip traffic sum(T(32 - min(group)))."""
    Is = sorted(bin_is)
    t = lambda m: m * (m + 1) // 2
    best = None

    def rec(i, acc, parts):
        nonlocal best
        if i == len(Is):
            if best is None or acc < best[0]:
                best = (acc, [list(p) for p in parts])
            return
        for g in range(1, 5):
            if i + g <= len(Is):
                rec(i + g, acc + t(NB - Is[i]), parts + [Is[i : i + g]])

    rec(0, 0, [])
    return best[1]


def _emit_core(nc, tc, pools, dram_io, core, mode, variant="full"):
    """K-major schedule: one row-block I at a time, full output row in PSUM
    (8 banks), K-sweep with each A-tile's weights amortized over all J-chunks
    (weight switches are ~180ns on HW; this gives 2 per (I,K) instead of 2
    per (I,K,phase))."""
    apool, bpool, cpool, psum_pool = pools
    import concourse.mybir as mybir

    f32 = mybir.dt.float32
    nplane = 2 if mode == "bf16x3" else 1
    dt_in = {
        "bf16x3": mybir.dt.bfloat16,
        "fp32r": mybir.dt.float32r,
        "fp32": f32,
    }[mode]
    apack, bcat, cpart = dram_io["apack"], dram_io["bcat"], dram_io["cpart"]
    bcat3 = bcat.rearrange("k (t n) -> k t n", t=nplane)

    bin_is = BINS[core]
    slot = {I: s for s, I in enumerate(sorted(bin_is))}
    bdma_engines = [nc.sync, nc.scalar]
    bdma_i = 0

    static_b = None
    if "nobdma" in variant:
        static_b = []
        for ci in range(NB // 8):
            sb_t = bpool.tile(
                [BLK, nplane, 2 * PHASE], dt_in, name=f"sb_{ci}", tag=f"sb{ci}", bufs=1
            )
            nc.gpsimd.memset(sb_t[:], 0.5)
            static_b.append(sb_t)

    for I in sorted(bin_is):
        nblk = NB - I
        base = ABASE[core][I]
        a_t = apool.tile([BLK, nblk, nplane, BLK], dt_in, name=f"a_{I}", tag="a")
        for j0 in range(0, nblk, ACHUNK):
            j1 = min(j0 + ACHUNK, nblk)
            nc.gpsimd.dma_start(
                a_t[:, j0:j1, :, :], apack[:, base + j0 : base + j1, :, :]
            )
        c0 = I // 4  # first active PSUM bank / J-chunk
        ps = {
            c: psum_pool.tile([BLK, PHASE], f32, name=f"ps_{I}_{c}", tag=f"ps{c}")
            for c in range(c0, NB // 4)
        }

        for K in range(I, NB):
            kb = K - I
            # B strip double-chunks (1024 cols -> 2KB DMA lines; hi+lo planes
            # in one DMA).  Each plane feeds two 512-wide matmuls (PSUM bank
            # limit).
            b_ts = {}
            for d in range(K // 8, NB // 8):
                pstart = max(K * BLK, 2 * PHASE * d)
                width = 2 * PHASE * (d + 1) - pstart
                if "nobdma" in variant:
                    b_ts[d] = (static_b[d], pstart, width)
                    continue
                b_t = bpool.tile(
                    [BLK, nplane, 2 * PHASE], dt_in, name=f"b_{K}_{d}", tag="b"
                )
                for t in range(nplane):
                    eng = bdma_engines[bdma_i % len(bdma_engines)]
                    bdma_i += 1
                    eng.dma_start(
                        b_t[:, t, :width],
                        bcat3[K * BLK : (K + 1) * BLK, t, pstart : pstart + width],
                    )
                b_ts[d] = (b_t, pstart, width)

            first = K == I
            if mode == "bf16x3":
                passes = [(0, 0), (0, 1), (1, 0)]  # (A plane, B plane)
            else:
                passes = [(0, 0)]
            for pi, (ta, tb) in enumerate(passes):
                a_w = a_t[:, kb, ta, :]
                for c in range(K // 4, NB // 4):
                    d = c // 2
                    b_t, pstart, width = b_ts[d]
                    cstart = max(pstart, PHASE * c)  # global col of this MM
                    cwidth = PHASE * (c + 1) - cstart
                    o = ps[c][:, cstart - PHASE * c : PHASE]
                    boff = cstart - pstart  # offset into the b tile
                    is_first = first and pi == 0
                    is_last = pi == len(passes) - 1 and K == min(4 * c + 3, NB - 1)
                    if "nomm" not in variant:
                        nc.tensor.matmul(
                            o, a_w, b_t[:, tb, boff : boff + cwidth],
                            start=is_first, stop=is_last,
                        )

        # Evict the full output row; next I's banks free up as copies drain.
        for c in range(c0, NB // 4):
            if "nomm" in variant and "noevict" in variant:
                continue
            coff0 = max(I * BLK - PHASE * c, 0)
            w = PHASE - coff0
            ct = cpool.tile([BLK, PHASE], f32, name=f"c_{I}_{c}", tag="cst")
            nc.vector.tensor_copy(ct[:, :w], ps[c][:, coff0:PHASE])
            r0 = slot[I] * BLK
            nc.gpsimd.dma_start(
                cpart[r0 : r0 + BLK, PHASE * c + coff0 : PHASE * (c + 1)],
                ct[:, :w],
            )


def _build(mode, repeat=1, variant="full"):
    import concourse.mybir as mybir
    import concourse.tile as tile
    from concourse import bacc

    nc = bacc.Bacc(None, target_bir_lowering=False, debug=False)
    f32 = mybir.dt.float32
    nplane = 2 if mode == "bf16x3" else 1
    dt_in = {
        "bf16x3": mybir.dt.bfloat16,
        "fp32r": mybir.dt.float32r,
        "fp32": f32,
    }[mode]
    with tile.TileContext(nc) as tc:
        with (
            tc.tile_pool(name="dram", bufs=1, space="DRAM") as dram,
            tc.tile_pool(name="apool", bufs=2) as apool,
            tc.tile_pool(name="bpool", bufs=16) as bpool,
            tc.tile_pool(name="cpool", bufs=4) as cpool,
            tc.tile_pool(name="psum", bufs=1, space="PSUM") as psum_pool,
        ):
            dram_io = {
                "apack": dram.tile(
                    [BLK, NSLOT, nplane, BLK], dt_in, kind="ExternalInput",
                    name="apack", uniquify=False,
                ),
                "bcat": dram.tile(
                    [N, nplane * N], dt_in, kind="ExternalInput",
                    name="bcat", uniquify=False,
                ),
                "cpart": dram.tile(
                    [MAXB * BLK, N], f32, kind="ExternalOutput",
                    name="cpart", uniquify=False,
                ),
            }
            pid = nc.partition_id()
            pools = (apool, bpool, cpool, psum_pool)
            for c in range(N_CORES):
                with tc.If(pid == c):
                    if repeat > 1:
                        with tc.For_i(
                            0, repeat, 1, hint_engines=tuple(mybir.ALL_ENGINES)
                        ):
                            _emit_core(nc, tc, pools, dram_io, c, mode, variant)
                    else:
                        _emit_core(nc, tc, pools, dram_io, c, mode, variant)
    nc.compile()
    return nc


_cached_nc = {}

# Optional profiling knobs (used by test.py; harness leaves them off).
TRACE = False
TRACE_KW = {}
LAST_RESULTS = None


def _get_nc(mode):
    if mode not in _cached_nc:
        _cached_nc[mode] = _build(mode)
    return _cached_nc[mode]


def _host_pack(A, B, mode):
    """Build per-core apack tensors and the shared bcat tensor."""
    if mode == "bf16x3":
        import ml_dtypes

        bf16 = ml_dtypes.bfloat16
        AT = np.ascontiguousarray(A.T)
        ath = AT.astype(bf16)
        atl = (AT - ath.astype(np.float32)).astype(bf16)
        bh_ = B.astype(bf16)
        bl_ = (B - bh_.astype(np.float32)).astype(bf16)
        planes_a = [ath, atl]
        bcat = np.concatenate([bh_, bl_], axis=1)
        npdt = bf16
    else:
        AT = np.ascontiguousarray(A.T)
        planes_a = [AT]
        bcat = np.ascontiguousarray(B)
        npdt = np.float32
    nplane = len(planes_a)

    apacks = []
    for c in range(N_CORES):
        ap = np.zeros((BLK, NSLOT, nplane, BLK), dtype=npdt)
        for I in BINS[c]:
            base = ABASE[c][I]
            for j, K in enumerate(range(I, NB)):
                for t, pl in enumerate(planes_a):
                    ap[:, base + j, t, :] = pl[
                        K * BLK : (K + 1) * BLK, I * BLK : (I + 1) * BLK
                    ]
        apacks.append(ap)
    return apacks, bcat


def kernel(A, B):
    from concourse.bass_utils import run_bass_kernel_spmd

    A = np.asarray(A, dtype=np.float32)
    B = np.asarray(B, dtype=np.float32)
    nc = _get_nc(MODE)
    apacks, bcat = _host_pack(A, B, MODE)
    in_maps = [{"apack": apacks[c], "bcat": bcat} for c in range(N_CORES)]
    res = run_bass_kernel_spmd(
        nc, in_maps, core_ids=list(range(N_CORES)), trace=TRACE, **TRACE_KW
    )
    global LAST_RESULTS
    LAST_RESULTS = res

    C = np.zeros((N, N), dtype=np.float32)
    for c in range(N_CORES):
        cp = res.results[c]["cpart"]
        for s, I in enumerate(sorted(BINS[c])):
            C[I * BLK : (I + 1) * BLK, I * BLK :] = cp[s * BLK : (s + 1) * BLK, I * BLK :]
    return C



# revision 9
# speedup vs baseline: 3.3047x; 3.3047x over previous
"""Triangular matmul C = triu(triu(A) @ triu(B)) on 8 TRN2 NeuronCores.

N=4096 fp32, viewed as a 32x32 grid of 128x128 blocks; the MAC work is the
block-tetrahedron {I <= K <= J} (5984 blocks of 128^3).

Sharding is 2D over the output: column "phases" (512-wide J-groups) are
split into two classes CLS = {0,3,5,6} / {1,2,4,7} carrying exactly half
the MACs each; within a class, rows go to 4 cores per ROWS_TBL (sets found
by local search balancing per-core max(compute, DMA) — ~748 MAC-blocks and
~13 MB of HBM traffic per core).  Core c = (row set c%4, class c//4).

Numerics: operands are rounded to bf16 on the host and each block product
is a single bf16 matmul accumulating in fp32 PSUM (rel err ~2e-3 vs the
fp32 reference; the harness gate is 2e-2).  C is staged to fp16 in SBUF
and upcast on the host.

Schedule per core: phases ascending.  Mid phases run K-strip-outer
(q = min_row..4p+3 ascending, one 512-wide matmul per owned row i <= q,
start at q == i, stop at q == 4p+3) so compute streams behind the panel
DMA; PSUM banks rotate through 8 tags so evictions overlap the next
phase.  The LAST phase runs row-outer with rows descending and its B
panel loaded in strip-descending chunks: each row's sweep completes as
soon as its strips land, and its eviction (DVE/ACT fp32->fp16 copy + a
HWDGE store) overlaps the remaining rows — no end-of-kernel eviction
cliff.  All loads are >=0.8 MB contiguous HWDGE DMAs from host-packed
per-core layouts; C stores are emitted after every load on the same
rings, so they can never block a load.  A dummy matmul burst at t=0
warms the PE clock gate, and a Switch computed-goto (with an early
prefetch hint) dispatches the 8 per-core programs without the ~40 us
serial If-chain walk.

The kernel takes FULL (unsharded) inputs and returns the FULL output.
"""

import numpy as np

N = 4096
BLK = 128
NB = 32
PW = 512  # phase width in cols (4 blocks) = one fp32 PSUM bank
N_CORES = 8
MODE = "bf16x1-2d-v3"

CLS = [[0, 3, 5, 6], [1, 2, 4, 7]]
# Row sets per class (4 cores each), from the assignment optimizer.
# (Rows 28-31 in class 0 have no class-0 output and emit nothing there.)
ROWS_TBL = [
    [[3, 7, 9, 14, 17, 20, 22, 24], [0, 6, 8, 10, 25, 26, 28, 31],
     [4, 11, 12, 13, 15, 16, 18, 23], [1, 2, 5, 19, 21, 27, 29, 30]],
    [[0, 1, 12, 24, 25, 26, 29, 31], [2, 3, 8, 13, 21, 22, 27, 30],
     [5, 7, 11, 14, 15, 16, 18, 20], [4, 6, 9, 10, 17, 19, 23, 28]],
]

A_CHUNK = 40  # A-load DMA granularity in slots (~1.25 MB)
B_CHUNK = 6144  # B-load DMA granularity in cols (~1.5 MB)
N_WARM = 12  # dummy warmup matmuls (beat the HAM clock gate)


def _core_rs(c):
    return c % 4, c // 4


def _rows_of(c):
    r, s = _core_rs(c)
    return ROWS_TBL[s][r]


def _phases(c):
    """[(p, active_rows)] for core c, skipping phases with no owned rows."""
    _, s = _core_rs(c)
    out = []
    for p in CLS[s]:
        act = sorted(i for i in _rows_of(c) if i <= 4 * p + 3)
        if act:
            out.append((p, act))
    return out


def _strips(p, m):
    """K-strips (q, col0, width_cols) of phase p starting at row-block m."""
    out = []
    for q in range(m, 4 * p + 4):
        c0 = max(4 * p, q) * BLK
        out.append((q, c0, (4 * p + 4) * BLK - c0))
    return out


def _b_layout(c):
    """{(p, q): within-bpack col offset}, total width, per-phase spans."""
    off, w = {}, 0
    spans = {}
    for p, act in _phases(c):
        p0 = w
        for q, _, wid in _strips(p, act[0]):
            off[(p, q)] = w
            w += wid
        spans[p] = (p0, w)
    return off, w, spans


def _a_layout(c):
    """Packed-A slots.  Mid-phase region ordered by (K, i); the last
    phase's exclusive strips (q > kprev) ordered rows-descending to match
    its row-outer consumption."""
    phs = _phases(c)
    rows = sorted(set(i for _, act in phs for i in act))
    kmax = 4 * phs[-1][0] + 3
    kprev = 4 * phs[-2][0] + 3 if len(phs) > 1 else -1
    slots = {}
    for q in range(0, kprev + 1):
        for i in rows:
            if i <= q:
                slots[(q, i)] = len(slots)
    for i in sorted(phs[-1][1], reverse=True):
        for q in range(max(i, kprev + 1), kmax + 1):
            slots[(q, i)] = len(slots)
    return slots


def _c_layout(c):
    """Packed-C 512-col slots: {(p, i): slot}."""
    slots = {}
    for p, act in _phases(c):
        for i in act:
            slots[(p, i)] = len(slots)
    return slots


NA_MAX = max(len(_a_layout(c)) for c in range(N_CORES))
WB_MAX = max(_b_layout(c)[1] for c in range(N_CORES))
NC_MAX = max(len(_c_layout(c)) for c in range(N_CORES))


def _emit_core(nc, tc, pools, dram_io, core):
    import concourse.mybir as mybir

    f32 = mybir.dt.float32
    bf16 = mybir.dt.bfloat16
    fp16 = mybir.dt.float16
    apool, bpool, cpool, psum_pool = pools
    apack, bpack, cpack = dram_io["apack"], dram_io["bpack"], dram_io["cpack"]
    aslot = _a_layout(core)
    cslot = _c_layout(core)
    boff, wb, bspans = _b_layout(core)
    phs = _phases(core)
    plast = phs[-1][0]
    kprev = 4 * phs[-2][0] + 3 if len(phs) > 1 else -1
    na = len(aslot)

    # --- PE warmup while the first loads are in flight.
    warm = bpool.tile([BLK, PW], bf16, name="warm", tag="warm")
    nc.gpsimd.memset(warm[:], 0.0)
    wps = psum_pool.tile([BLK, PW], f32, name="warmps", tag="ps7")
    for i in range(N_WARM):
        nc.tensor.matmul(
            wps[:], warm[:, :BLK], warm[:], start=(i == 0), stop=(i == N_WARM - 1)
        )

    # --- loads: A on the ACT ring, B on the SP ring, in consumption order.
    a_t = apool.tile([BLK, na, BLK], bf16, name="a", tag="a")
    b_t = {}
    prev_a = 0

    def load_a_upto(hi):
        nonlocal prev_a
        while prev_a < hi:
            step = min(A_CHUNK, hi - prev_a)
            nc.scalar.dma_start(
                a_t[:, prev_a : prev_a + step, :],
                apack[:, prev_a : prev_a + step, :],
            )
            prev_a += step

    for pi, (p, act) in enumerate(phs):
        p0, p1 = bspans[p]
        b_t[p] = bpool.tile([BLK, p1 - p0], bf16, name=f"b_{p}", tag=f"bp{pi}")
        if p != plast:
            load_a_upto(sum(1 for (q, _i) in aslot if q <= 4 * p + 3))
            w0 = p0
            while w0 < p1:
                step = min(B_CHUNK, p1 - w0)
                nc.sync.dma_start(
                    b_t[p][:, w0 - p0 : w0 - p0 + step], bpack[:, w0 : w0 + step]
                )
                w0 += step
        else:
            # strip-descending chunks to match the row-outer finale; A
            # slots for q > kprev are already laid out rows-descending.
            load_a_upto(na)
            chunks = []
            wlo = p1
            for q, _, wid in reversed(_strips(p, act[0])):
                wlo -= wid
                chunks.append((wlo, wid))
            accum = []
            acc_w = 0
            for wlo, wid in chunks:
                accum.append(wlo)
                acc_w += wid
                if acc_w >= B_CHUNK // 2:
                    lo = min(accum)
                    nc.sync.dma_start(
                        b_t[p][:, lo - p0 : lo - p0 + acc_w],
                        bpack[:, lo : lo + acc_w],
                    )
                    accum, acc_w = [], 0
            if accum:
                lo = min(accum)
                nc.sync.dma_start(
                    b_t[p][:, lo - p0 : lo - p0 + acc_w], bpack[:, lo : lo + acc_w]
                )

    # --- compute ---
    bank = 0
    store_eng = [nc.sync, nc.scalar]
    copy_eng = [nc.vector.tensor_copy, nc.scalar.copy]
    nstore = 0

    for p, act in phs:
        p0, _ = bspans[p]
        last_strip = 4 * p + 3
        if p != plast:
            # K-strip-outer: streams behind the ascending panel DMA.
            ps = {}
            for i in act:
                ps[i] = psum_pool.tile(
                    [BLK, PW], f32, name=f"ps_{p}_{i}", tag=f"ps{bank % 8}"
                )
                bank += 1
            for q, c0, wid in _strips(p, act[0]):
                rel = c0 - 4 * p * BLK
                for i in act:
                    if i > q:
                        continue
                    nc.tensor.matmul(
                        ps[i][:, rel : rel + wid],
                        a_t[:, aslot[(q, i)], :],
                        b_t[p][:, boff[(p, q)] - p0 : boff[(p, q)] - p0 + wid],
                        start=(q == i),
                        stop=(q == last_strip),
                    )
            cst = cpool.tile(
                [BLK, len(act) * PW], fp16, name=f"cst_{p}", tag=f"cst{p % 2}"
            )
            for j, i in enumerate(act):
                mr = max(0, i - 4 * p) * BLK
                nc.vector.tensor_copy(
                    cst[:, j * PW + mr : (j + 1) * PW], ps[i][:, mr:PW]
                )
            s0 = cslot[(p, act[0])]
            store_eng[nstore % 2].dma_start(
                cpack[:, s0 * PW : (s0 + len(act)) * PW], cst[:]
            )
            nstore += 1
        else:
            # Row-outer finale, rows descending: per-row eviction overlaps
            # the remaining rows' matmuls.
            for ji, i in enumerate(sorted(act, reverse=True)):
                pst = psum_pool.tile(
                    [BLK, PW], f32, name=f"ps_{p}_{i}", tag=f"ps{bank % 8}"
                )
                bank += 1
                for q, c0, wid in _strips(p, i):
                    rel = c0 - 4 * p * BLK
                    nc.tensor.matmul(
                        pst[:, rel : rel + wid],
                        a_t[:, aslot[(q, i)], :],
                        b_t[p][:, boff[(p, q)] - p0 : boff[(p, q)] - p0 + wid],
                        start=(q == i),
                        stop=(q == last_strip),
                    )
                mr = max(0, i - 4 * p) * BLK
                ct = cpool.tile([BLK, PW], fp16, name=f"ct_{i}", tag=f"ct{ji % 4}")
                copy_eng[ji % 2](ct[:, mr:PW], pst[:, mr:PW])
                store_eng[ji % 2].dma_start(
                    cpack[:, cslot[(p, i)] * PW + mr : (cslot[(p, i)] + 1) * PW],
                    ct[:, mr:PW],
                )


def _build():
    import concourse.mybir as mybir
    import concourse.tile as tile
    from concourse import bacc

    nc = bacc.Bacc(None, target_bir_lowering=False, debug=False)
    bf16 = mybir.dt.bfloat16
    fp16 = mybir.dt.float16
    with tile.TileContext(nc) as tc:
        with (
            tc.tile_pool(name="dram", bufs=1, space="DRAM") as dram,
            tc.tile_pool(name="apool", bufs=1) as apool,
            tc.tile_pool(name="bpool", bufs=1) as bpool,
            tc.tile_pool(name="cpool", bufs=1) as cpool,
            tc.tile_pool(name="psum", bufs=1, space="PSUM") as psum_pool,
        ):
            dram_io = {
                "apack": dram.tile(
                    [BLK, NA_MAX, BLK], bf16, kind="ExternalInput",
                    name="apack", uniquify=False,
                ),
                "bpack": dram.tile(
                    [BLK, WB_MAX], bf16, kind="ExternalInput",
                    name="bpack", uniquify=False,
                ),
                "cpack": dram.tile(
                    [BLK, NC_MAX * PW], fp16, kind="ExternalOutput",
                    name="cpack", uniquify=False,
                ),
            }
            pid = nc.partition_id()
            pools = (apool, bpool, cpool, psum_pool)
            tc.switch_hint({e: pid for e in mybir.ALL_ENGINES}, N_CORES, label="core")
            for c in tc.Switch(pid, N_CORES, hint="core"):
                _emit_core(nc, tc, pools, dram_io, c)
    nc.compile()
    return nc


_cached_nc = None

# Optional profiling knobs (used by test.py; harness leaves them off).
TRACE = False
TRACE_KW = {}
LAST_RESULTS = None


def _get_nc():
    global _cached_nc
    if _cached_nc is None:
        _cached_nc = _build()
    return _cached_nc


def _host_pack(A, B):
    import ml_dtypes

    bf16 = ml_dtypes.bfloat16
    AT = np.ascontiguousarray(A.T).astype(bf16)
    Bb = B.astype(bf16)
    apacks, bpacks = [], []
    for c in range(N_CORES):
        ap = np.zeros((BLK, NA_MAX, BLK), dtype=bf16)
        for (q, i), idx in _a_layout(c).items():
            ap[:, idx, :] = AT[q * BLK : (q + 1) * BLK, i * BLK : (i + 1) * BLK]
        bp = np.zeros((BLK, WB_MAX), dtype=bf16)
        boff, _, _ = _b_layout(c)
        for p, act in _phases(c):
            for q, c0, wid in _strips(p, act[0]):
                w0 = boff[(p, q)]
                bp[:, w0 : w0 + wid] = Bb[q * BLK : (q + 1) * BLK, c0 : c0 + wid]
        apacks.append(ap)
        bpacks.append(bp)
    return apacks, bpacks


def kernel(A, B):
    from concourse.bass_utils import run_bass_kernel_spmd

    A = np.asarray(A, dtype=np.float32)
    B = np.asarray(B, dtype=np.float32)
    nc = _get_nc()
    apacks, bpacks = _host_pack(A, B)
    in_maps = [{"apack": apacks[c], "bpack": bpacks[c]} for c in range(N_CORES)]
    res = run_bass_kernel_spmd(
        nc, in_maps, core_ids=list(range(N_CORES)), trace=TRACE, **TRACE_KW
    )
    global LAST_RESULTS
    LAST_RESULTS = res

    C = np.zeros((N, N), dtype=np.float32)
    for c in range(N_CORES):
        cp = res.results[c]["cpack"]
        for (p, i), j in _c_layout(c).items():
            mr = max(0, i - 4 * p) * BLK
            C[i * BLK : (i + 1) * BLK, p * PW + mr : (p + 1) * PW] = cp[
                :, j * PW + mr : (j + 1) * PW
            ].astype(np.float32)
    return np.triu(C)
